# revision 5
# baseline (speedup 1.0000x reference)
"""VQ codebook-lookup kernel for Trainium2 (8 NeuronCores, data-parallel over batch).

For each (batch, head, token): find nearest codebook row (L2) among 2048 codes,
output that codebook row. argmin ||q - c||^2 == argmax (q.c - 0.5||c||^2).

Scoring runs at a 2^12 scale (argmax is scale-invariant), which lets the two
fp16 hi/lo correction matmuls of the naive scheme collapse into ONE fp8
DoubleRow matmul (2 fp8 products per PE cell per cycle):

  q.c = qh.ch + q.cl + ql.ch          (exact identity, qh=fp16(q), ql=q-qh)
  leading:    fp16(q*2^6) . fp16(c*2^6)              -> q.c*2^12 (fp16 matmul)
  correction: e4m3(q).e4m3(cl*2^12) + e4m3(ql*2^12).e4m3(ch)   (one DR matmul)

Numpy-validated on the reference data: 2/131072 argmin flips, rel err 5e-3.

Per core (one batch of 8):
  - fused custom DVE op does bias-add (-0.5||c||^2*2^12) + running-max scan +
    argmax-index extraction in ONE 1x pass straight from PSUM
  - GPSIMD indirect DMA gathers the winning codebook rows from DRAM,
    assembled into [128, 1024] out tiles and streamed out during the
    last head's pass
Host side pre-transposes/splits/scales operands so no on-chip transposes
or conversions are needed.
"""

import numpy as np
import ml_dtypes

import concourse.mybir as mybir
import concourse.tile as tile
from concourse import bacc
from concourse.bass import IndirectOffsetOnAxis
from concourse.bass_utils import run_bass_kernel_spmd

# problem constants (hardcoded per contract)
B = 8  # batch (== n_cores, data-parallel)
N = 2048  # tokens per batch
H = 8  # heads
D = 128  # head dim
M = 2048  # codebook size
NT = N // 128  # 16 n-tiles per head
MB = 4  # m-blocks of 512 per matmul set

SC6 = np.float32(64.0)  # operand scale for the fp16 leading matmul
SC12 = np.float32(4096.0)  # score scale (= SC6*SC6); fp8 low parts are scaled by it

f32 = mybir.dt.float32
f16 = mybir.dt.float16
f8 = mybir.dt.float8e4
i32 = mybir.dt.int32
bf16 = mybir.dt.bfloat16

E4 = ml_dtypes.float8_e4m3

# ---------------------------------------------------------------------------
# custom DVE op: one-pass fused (bias-add, running-max scan, argmax index)
# ---------------------------------------------------------------------------
_ARGMAX_OP = None


def _get_argmax_op():
    global _ARGMAX_OP
    if _ARGMAX_OP is not None:
        return _ARGMAX_OP
    import concourse.dve_ops as dve_ops_mod
    from concourse.dve_ops import CUSTOM_DVE_SPECS, OPS, DveOp
    from concourse.dve_spec import (
        AluOp,
        Idx,
        MaxNeg,
        One,
        Spec,
        Src0,
        Src1,
        Zero,
        eq,
        lower,
        maxx,
        scan,
        select,
    )
    from concourse.dve_uop import DveOpSpec

    name = "ARGMAX_BIAS_ANT"
    for existing in OPS:
        if existing.name == name:  # already registered in this process
            _ARGMAX_OP = existing
            return existing

    def _ref(in0, in1, s0, s1, imm2):
        s = in0.astype(np.float32) + in1.astype(np.float32)
        m = np.maximum.accumulate(s, axis=-1)
        idx = np.arange(s.shape[-1], dtype=np.float32)
        fired = np.where(s == m, idx, -1.0).astype(np.float32)
        acc = fired.max(axis=-1).reshape(s.shape[0], 1).astype(np.float32)
        return fired, acc

    s = Src0 + Src1
    m = scan(AluOp.MAX, s)
    body = select(eq(s, m), Idx, Zero - One)
    spec = Spec(body=body, accum=maxx, accum_init=MaxNeg, reference=_ref)
    shas = {}
    for ver in ("v3", "v4"):
        ups = lower(spec, ver=ver)
        shas[ver] = DveOpSpec(name=name, opcode=0, uops=ups, rd1_en=True).sha(ver)
    op = DveOp(name, spec, subdim=False, uops_sha=shas)
    OPS.append(op)
    CUSTOM_DVE_SPECS[name] = spec
    dve_ops_mod._SUB_OPCODE_FOR_NAME[name] = (
        dve_ops_mod._CUSTOM_DVE_ROW_BASE + len(OPS) - 1
    )
    _ARGMAX_OP = op
    return op


# ---------------------------------------------------------------------------
# bass kernel builder
# ---------------------------------------------------------------------------
_NC_CACHE = None


def _build_nc():
    global _NC_CACHE
    if _NC_CACHE is not None:
        return _NC_CACHE
    argmax_op = _get_argmax_op()

    nc = bacc.Bacc("TRN2", target_bir_lowering=False, debug=False, num_devices=B)

    # DRAM I/O (per-core views; each core gets its own batch slice of q)
    d_qh = nc.dram_tensor("qh", [H, D, N], f16, kind="ExternalInput")
    d_qp = nc.dram_tensor("qp", [H, D, 2, N], f8, kind="ExternalInput")
    d_ch = nc.dram_tensor("ch", [H, D, M], f16, kind="ExternalInput")
    d_cp = nc.dram_tensor("cp", [H, D, 2, M], f8, kind="ExternalInput")
    d_c2 = nc.dram_tensor("c2bc", [H, 128, M], f32, kind="ExternalInput")
    d_cb = nc.dram_tensor("cb", [H * M, D], f32, kind="ExternalInput")
    d_out = nc.dram_tensor("out", [N, H * D], f32, kind="ExternalOutput")

    DR = mybir.MatmulPerfMode.DoubleRow

    with tile.TileContext(nc) as tc:
        with (
            tc.tile_pool(name="heads", bufs=2) as hp,
            tc.tile_pool(name="outs", bufs=1) as op_pool,
            tc.tile_pool(name="small", bufs=2) as sp,
            tc.tile_pool(name="scr", bufs=1) as scrp,
            tc.tile_pool(name="ps", bufs=2, space="PSUM") as ps,
        ):
            out_tiles = []
            for t in range(NT):
                ot = op_pool.tile([128, H * D], f32, tag=f"out{t}")
                out_tiles.append(ot)
            scratch = scrp.tile([128, M], bf16, tag="scratch")

            # HAM warm-up: ~5us of dummy matmuls on a zeroed tile fills the
            # preamble->first-data window so real matmuls start at 2.4 GHz
            wz = scrp.tile([128, 512], f16, tag="warmz")
            nc.gpsimd.memset(wz[:], 0)
            psw = ps.tile([128, M], f32, tag="scores")
            for r in range(12):
                nc.tensor.matmul(
                    psw[:, 0:512], wz[:, 0:128], wz[:], start=True, stop=True
                )

            for h in range(H):
                s_qh = hp.tile([D, N], f16, tag="qh")
                s_qp = hp.tile([D, 2, N], f8, tag="qp")
                s_ch = hp.tile([D, M], f16, tag="ch")
                s_cp = hp.tile([D, 2, M], f8, tag="cp")
                s_c2 = hp.tile([128, M], f32, tag="c2")
                # order: first matmul needs qh+ch; fp8 packs follow; c2 only
                # at argmax. Spread across both HWDGE rings (sync + scalar).
                if h == 0:
                    # h0 is latency-critical: the first argmax needs the q
                    # tile-0 slices (tiny), ch+cp in full, and c2 — in that
                    # order. Defer the qh/qp remainders until after c2.
                    nc.sync.dma_start(s_qh[:, 0:128], d_qh[h][:, 0:128])
                    nc.scalar.dma_start(s_qp[:, :, 0:128], d_qp[h][:, :, 0:128])
                    nc.sync.dma_start(s_ch[:], d_ch[h])
                    nc.scalar.dma_start(s_cp[:], d_cp[h])
                    nc.sync.dma_start(s_c2[:, 0 : M // 2], d_c2[h][:, 0 : M // 2])
                    nc.scalar.dma_start(s_c2[:, M // 2 :], d_c2[h][:, M // 2 :])
                    nc.sync.dma_start(s_qh[:, 128:], d_qh[h][:, 128:])
                    nc.scalar.dma_start(s_qp[:, :, 128:], d_qp[h][:, :, 128:])
                else:
                    nc.sync.dma_start(s_qh[:], d_qh[h])
                    nc.scalar.dma_start(s_ch[:], d_ch[h])
                    nc.sync.dma_start(s_cp[:], d_cp[h])
                    nc.scalar.dma_start(s_qp[:], d_qp[h])
                    # c2 split across both rings so the argmax isn't gated
                    # by a single 1MB transfer
                    nc.sync.dma_start(s_c2[:, 0 : M // 2], d_c2[h][:, 0 : M // 2])
                    nc.scalar.dma_start(s_c2[:, M // 2 :], d_c2[h][:, M // 2 :])

                idx_f = sp.tile([128, NT], f32, tag="idxf")
                idx_i = sp.tile([128, NT], i32, tag="idxi")

                for t in range(NT):
                    psc = ps.tile([128, M], f32, tag="scores")
                    qh_t = s_qh[:, t * 128 : (t + 1) * 128]
                    qp_t = s_qp[:, :, t * 128 : (t + 1) * 128]
                    # leading fp16 blocks (start), then one fp8 DoubleRow
                    # correction matmul per block (stop)
                    for kblk in range(MB):
                        blk = slice(kblk * 512, (kblk + 1) * 512)
                        nc.tensor.matmul(
                            psc[:, blk], qh_t, s_ch[:, blk], start=True, stop=False
                        )
                    for kblk in range(MB):
                        blk = slice(kblk * 512, (kblk + 1) * 512)
                        nc.tensor.matmul(
                            psc[:, blk],
                            qp_t,
                            s_cp[:, :, blk],
                            start=False,
                            stop=True,
                            perf_mode=DR,
                        )
                    # fused bias-add + argmax over m=2048, one DVE pass
                    # (accum_out must be f32: dve_read_accumulator_type_check)
                    nc.vector._custom_dve(
                        argmax_op,
                        out=scratch[:],
                        in0=psc[:],
                        in1=s_c2[:],
                        accum_out=idx_f[:, t : t + 1],
                    )
                    # cast f32 index -> i32 on the (idle) scalar engine,
                    # then gather this tile's codebook rows immediately
                    nc.scalar.copy(idx_i[:, t : t + 1], idx_f[:, t : t + 1])
                    nc.gpsimd.indirect_dma_start(
                        out=out_tiles[t][:, h * D : (h + 1) * D],
                        out_offset=None,
                        in_=d_cb[:],
                        in_offset=IndirectOffsetOnAxis(ap=idx_i[:, t : t + 1], axis=0),
                        element_offset=h * M * D,
                    )
                    if h == H - 1:
                        # tile complete after the last head's gather: stream out
                        nc.sync.dma_start(
                            d_out[t * 128 : (t + 1) * 128, :], out_tiles[t][:]
                        )

    nc.compile()
    _NC_CACHE = nc
    return nc


# ---------------------------------------------------------------------------
# host wrapper
# ---------------------------------------------------------------------------


def _prepare_inputs(x, codebooks):
    x = np.ascontiguousarray(np.asarray(x, dtype=np.float32))
    cb = np.ascontiguousarray(np.asarray(codebooks, dtype=np.float32))

    # q transposed per (batch, head): [B, H, D, N]
    qT = np.ascontiguousarray(x.reshape(B, N, H, D).transpose(0, 2, 3, 1))
    qh16 = (qT * SC6).astype(np.float16)
    qh = qh16.astype(np.float32) / SC6
    ql = qT - qh
    # fp8 pack for DoubleRow: [B, H, D, 2, N]: [...,0,:]=e4m3(q), [...,1,:]=e4m3(ql*2^12)
    qp = np.empty((B, H, D, 2, N), dtype=E4)
    qp[:, :, :, 0, :] = qT.astype(E4)
    qp[:, :, :, 1, :] = (ql * SC12).astype(E4)

    # codebooks transposed per head: [H, D, M]
    cT = np.ascontiguousarray(cb.transpose(0, 2, 1))
    ch16 = (cT * SC6).astype(np.float16)
    ch = ch16.astype(np.float32) / SC6
    cl = cT - ch
    # fp8 pack: [H, D, 2, M]: [...,0,:]=e4m3(cl*2^12), [...,1,:]=e4m3(ch)
    cp = np.empty((H, D, 2, M), dtype=E4)
    cp[:, :, 0, :] = (cl * SC12).astype(E4)
    cp[:, :, 1, :] = ch.astype(E4)

    # -0.5 * ||c||^2 * 2^12 broadcast to 128 partitions: [H, 128, M]
    c2 = -0.5 * (cb.astype(np.float64) ** 2).sum(-1) * float(SC12)  # [H, M]
    c2bc = np.ascontiguousarray(
        np.broadcast_to(c2.astype(np.float32)[:, None, :], (H, 128, M))
    )

    cb_flat = np.ascontiguousarray(cb.reshape(H * M, D))

    shared = {
        "ch": np.ascontiguousarray(ch16),
        "cp": np.ascontiguousarray(cp),
        "c2bc": c2bc,
        "cb": cb_flat,
    }
    in_maps = []
    for b in range(B):
        m = dict(shared)
        m["qh"] = np.ascontiguousarray(qh16[b])
        m["qp"] = np.ascontiguousarray(qp[b])
        in_maps.append(m)
    return in_maps


_LAST_RESULTS = None  # stashed for test harness (exec time inspection)


def kernel(x, codebooks, _trace=False, _trace_kwargs=None):
    global _LAST_RESULTS
    import os

    nc = _build_nc()
    in_maps = _prepare_inputs(x, codebooks)
    kw = {}
    if _trace:
        kw["trace"] = True
        kw.update(_trace_kwargs or {})
    else:
        # without the axon NTFF hook installed, a stray BASS_TRACE env would
        # crash run_bass_kernel_spmd on a missing antenv.axon_hooks import
        os.environ["BASS_NEVER_TRACE"] = "1"
    res = run_bass_kernel_spmd(nc, in_maps, core_ids=list(range(B)), **kw)
    if not _trace:
        os.environ.pop("BASS_NEVER_TRACE", None)
    _LAST_RESULTS = res
    out = np.stack([res.results[b]["out"] for b in range(B)], axis=0)
    return out.astype(np.float32)


# revision 6
# speedup vs baseline: 1.0063x; 1.0063x over previous
"""VQ codebook-lookup kernel for Trainium2 (8 NeuronCores, data-parallel over batch).

For each (batch, head, token): find nearest codebook row (L2) among 2048 codes,
output that codebook row. argmin ||q - c||^2 == argmax (q.c - 0.5||c||^2).

Scoring runs at a 2^12 scale (argmax is scale-invariant), which lets the two
fp16 hi/lo correction matmuls of the naive scheme collapse into ONE fp8
DoubleRow matmul (2 fp8 products per PE cell per cycle):

  q.c = qh.ch + q.cl + ql.ch          (exact identity, qh=fp16(q), ql=q-qh)
  leading:    fp16(q*2^6) . fp16(c*2^6)              -> q.c*2^12 (fp16 matmul)
  correction: e4m3(q).e4m3(cl*2^12) + e4m3(ql*2^12).e4m3(ch)   (one DR matmul)

Numpy-validated on the reference data: 2/131072 argmin flips, rel err 5e-3.

Per core (one batch of 8):
  - fused custom DVE op does bias-add (-0.5||c||^2*2^12) + running-max scan +
    argmax-index extraction in ONE 1x pass straight from PSUM
  - GPSIMD indirect DMA gathers the winning codebook rows from DRAM,
    assembled into [128, 1024] out tiles and streamed out during the
    last head's pass
Host side pre-transposes/splits/scales operands so no on-chip transposes
or conversions are needed.
"""

import numpy as np
import ml_dtypes

import concourse.mybir as mybir
import concourse.tile as tile
from concourse import bacc
from concourse.bass import IndirectOffsetOnAxis
from concourse.bass_utils import run_bass_kernel_spmd

# problem constants (hardcoded per contract)
B = 8  # batch (== n_cores, data-parallel)
N = 2048  # tokens per batch
H = 8  # heads
D = 128  # head dim
M = 2048  # codebook size
NT = N // 128  # 16 n-tiles per head
MB = 4  # m-blocks of 512 per matmul set

SC6 = np.float32(64.0)  # operand scale for the fp16 leading matmul
SC12 = np.float32(4096.0)  # score scale (= SC6*SC6); fp8 low parts are scaled by it

f32 = mybir.dt.float32
f16 = mybir.dt.float16
f8 = mybir.dt.float8e4
i32 = mybir.dt.int32
bf16 = mybir.dt.bfloat16

E4 = ml_dtypes.float8_e4m3

# ---------------------------------------------------------------------------
# custom DVE op: one-pass fused (bias-add, running-max scan, argmax index)
# ---------------------------------------------------------------------------
_ARGMAX_OP = None


def _get_argmax_op():
    global _ARGMAX_OP
    if _ARGMAX_OP is not None:
        return _ARGMAX_OP
    import concourse.dve_ops as dve_ops_mod
    from concourse.dve_ops import CUSTOM_DVE_SPECS, OPS, DveOp
    from concourse.dve_spec import (
        AluOp,
        Idx,
        MaxNeg,
        One,
        Spec,
        Src0,
        Src1,
        Zero,
        eq,
        lower,
        maxx,
        scan,
        select,
    )
    from concourse.dve_uop import DveOpSpec

    name = "ARGMAX_BIAS_ANT"
    for existing in OPS:
        if existing.name == name:  # already registered in this process
            _ARGMAX_OP = existing
            return existing

    def _ref(in0, in1, s0, s1, imm2):
        s = in0.astype(np.float32) + in1.astype(np.float32)
        m = np.maximum.accumulate(s, axis=-1)
        idx = np.arange(s.shape[-1], dtype=np.float32)
        fired = np.where(s == m, idx, -1.0).astype(np.float32)
        acc = fired.max(axis=-1).reshape(s.shape[0], 1).astype(np.float32)
        return fired, acc

    s = Src0 + Src1
    m = scan(AluOp.MAX, s)
    body = select(eq(s, m), Idx, Zero - One)
    spec = Spec(body=body, accum=maxx, accum_init=MaxNeg, reference=_ref)
    shas = {}
    for ver in ("v3", "v4"):
        ups = lower(spec, ver=ver)
        shas[ver] = DveOpSpec(name=name, opcode=0, uops=ups, rd1_en=True).sha(ver)
    op = DveOp(name, spec, subdim=False, uops_sha=shas)
    OPS.append(op)
    CUSTOM_DVE_SPECS[name] = spec
    dve_ops_mod._SUB_OPCODE_FOR_NAME[name] = (
        dve_ops_mod._CUSTOM_DVE_ROW_BASE + len(OPS) - 1
    )
    _ARGMAX_OP = op
    return op


# ---------------------------------------------------------------------------
# bass kernel builder
# ---------------------------------------------------------------------------
_NC_CACHE = None


def _build_nc():
    global _NC_CACHE
    if _NC_CACHE is not None:
        return _NC_CACHE
    argmax_op = _get_argmax_op()

    nc = bacc.Bacc("TRN2", target_bir_lowering=False, debug=False, num_devices=B)

    # DRAM I/O (per-core views; each core gets its own batch slice of q)
    d_qh = nc.dram_tensor("qh", [H, D, N], f16, kind="ExternalInput")
    d_qp = nc.dram_tensor("qp", [H, D, 2, N], f8, kind="ExternalInput")
    d_ch = nc.dram_tensor("ch", [H, D, M], f16, kind="ExternalInput")
    d_cp = nc.dram_tensor("cp", [H, D, 2, M], f8, kind="ExternalInput")
    d_c2 = nc.dram_tensor("c2bc", [H, 128, M], f32, kind="ExternalInput")
    d_cb = nc.dram_tensor("cb", [H * M, D], f32, kind="ExternalInput")
    d_out = nc.dram_tensor("out", [N, H * D], f32, kind="ExternalOutput")

    DR = mybir.MatmulPerfMode.DoubleRow

    with tile.TileContext(nc) as tc:
        with (
            tc.tile_pool(name="heads", bufs=2) as hp,
            tc.tile_pool(name="outs", bufs=1) as op_pool,
            tc.tile_pool(name="small", bufs=2) as sp,
            tc.tile_pool(name="scr", bufs=1) as scrp,
            tc.tile_pool(name="ps", bufs=2, space="PSUM") as ps,
        ):
            out_tiles = []
            for t in range(NT):
                ot = op_pool.tile([128, H * D], f32, tag=f"out{t}")
                out_tiles.append(ot)
            scratch = scrp.tile([128, M], bf16, tag="scratch")

            # HAM warm-up: ~5us of dummy matmuls on a zeroed tile fills the
            # preamble->first-data window so real matmuls start at 2.4 GHz
            wz = scrp.tile([128, 512], f16, tag="warmz")
            nc.gpsimd.memset(wz[:], 0)
            psw = ps.tile([128, M], f32, tag="scores")
            for r in range(12):
                nc.tensor.matmul(
                    psw[:, 0:512], wz[:, 0:128], wz[:], start=True, stop=True
                )

            for h in range(H):
                s_qh = hp.tile([D, N], f16, tag="qh")
                s_qp = hp.tile([D, 2, N], f8, tag="qp")
                s_ch = hp.tile([D, M], f16, tag="ch")
                s_cp = hp.tile([D, 2, M], f8, tag="cp")
                s_c2 = hp.tile([128, M], f32, tag="c2")
                # order: first matmul needs qh+ch; fp8 packs follow; c2 only
                # at argmax. Spread across both HWDGE rings (sync + scalar).
                if h == 0:
                    # h0 is latency-critical: the first argmax needs the q
                    # tile-0 slices (tiny), ch+cp in full, and c2. Ship c2
                    # on the (idle at startup) gpsimd SWDGE ring so the two
                    # HWDGE rings carry only q/c data in tile order.
                    nc.sync.dma_start(s_qh[:, 0:128], d_qh[h][:, 0:128])
                    nc.scalar.dma_start(s_qp[:, :, 0:128], d_qp[h][:, :, 0:128])
                    nc.gpsimd.dma_start(s_c2[:, 0 : M // 2], d_c2[h][:, 0 : M // 2])
                    nc.gpsimd.dma_start(s_c2[:, M // 2 :], d_c2[h][:, M // 2 :])
                    nc.sync.dma_start(s_ch[:], d_ch[h])
                    nc.scalar.dma_start(s_cp[:], d_cp[h])
                    nc.sync.dma_start(s_qh[:, 128:], d_qh[h][:, 128:])
                    nc.scalar.dma_start(s_qp[:, :, 128:], d_qp[h][:, :, 128:])
                else:
                    nc.sync.dma_start(s_qh[:], d_qh[h])
                    nc.scalar.dma_start(s_ch[:], d_ch[h])
                    nc.sync.dma_start(s_cp[:], d_cp[h])
                    nc.scalar.dma_start(s_qp[:], d_qp[h])
                    # c2 split across both rings so the argmax isn't gated
                    # by a single 1MB transfer
                    nc.sync.dma_start(s_c2[:, 0 : M // 2], d_c2[h][:, 0 : M // 2])
                    nc.scalar.dma_start(s_c2[:, M // 2 :], d_c2[h][:, M // 2 :])

                idx_f = sp.tile([128, NT], f32, tag="idxf")
                idx_i = sp.tile([128, NT], i32, tag="idxi")

                for t in range(NT):
                    psc = ps.tile([128, M], f32, tag="scores")
                    qh_t = s_qh[:, t * 128 : (t + 1) * 128]
                    qp_t = s_qp[:, :, t * 128 : (t + 1) * 128]
                    # leading fp16 blocks (start), then one fp8 DoubleRow
                    # correction matmul per block (stop)
                    for kblk in range(MB):
                        blk = slice(kblk * 512, (kblk + 1) * 512)
                        nc.tensor.matmul(
                            psc[:, blk], qh_t, s_ch[:, blk], start=True, stop=False
                        )
                    for kblk in range(MB):
                        blk = slice(kblk * 512, (kblk + 1) * 512)
                        nc.tensor.matmul(
                            psc[:, blk],
                            qp_t,
                            s_cp[:, :, blk],
                            start=False,
                            stop=True,
                            perf_mode=DR,
                        )
                    # fused bias-add + argmax over m=2048, one DVE pass
                    # (accum_out must be f32: dve_read_accumulator_type_check)
                    nc.vector._custom_dve(
                        argmax_op,
                        out=scratch[:],
                        in0=psc[:],
                        in1=s_c2[:],
                        accum_out=idx_f[:, t : t + 1],
                    )
                    # cast f32 index -> i32 on the (idle) scalar engine,
                    # then gather this tile's codebook rows immediately
                    nc.scalar.copy(idx_i[:, t : t + 1], idx_f[:, t : t + 1])
                    nc.gpsimd.indirect_dma_start(
                        out=out_tiles[t][:, h * D : (h + 1) * D],
                        out_offset=None,
                        in_=d_cb[:],
                        in_offset=IndirectOffsetOnAxis(ap=idx_i[:, t : t + 1], axis=0),
                        element_offset=h * M * D,
                    )
                    if h == H - 1:
                        # tile complete after the last head's gather: stream out
                        nc.sync.dma_start(
                            d_out[t * 128 : (t + 1) * 128, :], out_tiles[t][:]
                        )

    nc.compile()
    _NC_CACHE = nc
    return nc


# ---------------------------------------------------------------------------
# host wrapper
# ---------------------------------------------------------------------------


def _prepare_inputs(x, codebooks):
    x = np.ascontiguousarray(np.asarray(x, dtype=np.float32))
    cb = np.ascontiguousarray(np.asarray(codebooks, dtype=np.float32))

    # q transposed per (batch, head): [B, H, D, N]
    qT = np.ascontiguousarray(x.reshape(B, N, H, D).transpose(0, 2, 3, 1))
    qh16 = (qT * SC6).astype(np.float16)
    qh = qh16.astype(np.float32) / SC6
    ql = qT - qh
    # fp8 pack for DoubleRow: [B, H, D, 2, N]: [...,0,:]=e4m3(q), [...,1,:]=e4m3(ql*2^12)
    qp = np.empty((B, H, D, 2, N), dtype=E4)
    qp[:, :, :, 0, :] = qT.astype(E4)
    qp[:, :, :, 1, :] = (ql * SC12).astype(E4)

    # codebooks transposed per head: [H, D, M]
    cT = np.ascontiguousarray(cb.transpose(0, 2, 1))
    ch16 = (cT * SC6).astype(np.float16)
    ch = ch16.astype(np.float32) / SC6
    cl = cT - ch
    # fp8 pack: [H, D, 2, M]: [...,0,:]=e4m3(cl*2^12), [...,1,:]=e4m3(ch)
    cp = np.empty((H, D, 2, M), dtype=E4)
    cp[:, :, 0, :] = (cl * SC12).astype(E4)
    cp[:, :, 1, :] = ch.astype(E4)

    # -0.5 * ||c||^2 * 2^12 broadcast to 128 partitions: [H, 128, M]
    c2 = -0.5 * (cb.astype(np.float64) ** 2).sum(-1) * float(SC12)  # [H, M]
    c2bc = np.ascontiguousarray(
        np.broadcast_to(c2.astype(np.float32)[:, None, :], (H, 128, M))
    )

    cb_flat = np.ascontiguousarray(cb.reshape(H * M, D))

    shared = {
        "ch": np.ascontiguousarray(ch16),
        "cp": np.ascontiguousarray(cp),
        "c2bc": c2bc,
        "cb": cb_flat,
    }
    in_maps = []
    for b in range(B):
        m = dict(shared)
        m["qh"] = np.ascontiguousarray(qh16[b])
        m["qp"] = np.ascontiguousarray(qp[b])
        in_maps.append(m)
    return in_maps


_LAST_RESULTS = None  # stashed for test harness (exec time inspection)


def kernel(x, codebooks, _trace=False, _trace_kwargs=None):
    global _LAST_RESULTS
    import os

    nc = _build_nc()
    in_maps = _prepare_inputs(x, codebooks)
    kw = {}
    if _trace:
        kw["trace"] = True
        kw.update(_trace_kwargs or {})
    else:
        # without the axon NTFF hook installed, a stray BASS_TRACE env would
        # crash run_bass_kernel_spmd on a missing antenv.axon_hooks import
        os.environ["BASS_NEVER_TRACE"] = "1"
    res = run_bass_kernel_spmd(nc, in_maps, core_ids=list(range(B)), **kw)
    if not _trace:
        os.environ.pop("BASS_NEVER_TRACE", None)
    _LAST_RESULTS = res
    out = np.stack([res.results[b]["out"] for b in range(B)], axis=0)
    return out.astype(np.float32)
